# revision 2
# baseline (speedup 1.0000x reference)
"""KAN layer (silu residual + cubic B-spline mixing) on 8 Trainium2 cores.

Math: out(b,o) = sum_i [ scale_base*mask*silu(x) + scale_sp*mask*sum_j N_j(x)*coef ]
The grid is uniform, so the 35 spline basis functions are shifts of ONE
cubic bump N3 on knots {0..4}:  bases_j(b,i) = N3(d(b,i) - j),
d = (x - t0)/h.  Cancellation-free closed form used on-device:

    p  = relu(2 - |d - j - 2|)          (custom DVE op A, PageIdx supplies j)
    6*N3 = p^3 - 4*relu(p-1)^3          (custom DVE op B)

The whole layer then becomes one dense matmul with contraction over
(i, j) [8960] plus a silu-residual contraction over i [256]:
    out = S * [ sum_{i,j} B6[(i,j),b]^T @ coef[(i,j),o] + silu_scaled^T @ scale_base ]

Sharding: data-parallel over batch; 8 cores x 256 rows each. coef and
scale_base replicated. Inside each core: i on partitions for the basis
build (lhsT layout, no transposes anywhere), PSUM accumulates (b,o).
"""

import sys

if "/opt/trn_rl_repo" not in sys.path:
    sys.path.insert(0, "/opt/trn_rl_repo")

import numpy as np

# ----------------------------------------------------------------- constants
P = 128
IN_DIM = 256
OUT_DIM = 512
BATCH = 2048
N_CORES = 8
B_LOC = BATCH // N_CORES  # 256
NJ = 35  # spline basis functions (G + K)
GJ = 7  # j-chunk size for DVE/PE pipelining
N_CHUNK = NJ // GJ  # 5
N_IT = IN_DIM // P  # 2 partition tiles of in_dim
N_M = B_LOC // P  # 2 output row tiles

# ------------------------------------------------------- custom DVE ops
_OPS_CACHE = {}


def _get_ops():
    if _OPS_CACHE:
        return _OPS_CACHE["A"], _OPS_CACHE["B"]
    import concourse.dve_ops as dve_ops
    from concourse.dve_ops import DveOp, OPS
    from concourse.dve_spec import (
        C0,
        C1,
        C2,
        PageIdx,
        Spec,
        Src0,
        Zero,
        One,
        _has_src1,
        lower,
        maxx,
        relu,
        sq,
    )
    from concourse.dve_uop import DveOpSpec

    def _register(name, spec, subdim):
        for op in OPS:
            if op.name == name:
                return op
        shas = {}
        for ver in ("v3", "v4"):
            tmp = DveOpSpec(
                name=name, opcode=1, uops=lower(spec, ver=ver), rd1_en=_has_src1(spec)
            )
            shas[ver] = tmp.sha(ver)
        op = DveOp(name, spec, subdim=subdim, uops_sha=shas)
        OPS.append(op)
        dve_ops.CUSTOM_DVE_SPECS[name] = spec
        dve_ops._SUB_OPCODE_FOR_NAME[name] = (
            dve_ops._CUSTOM_DVE_ROW_BASE + len(OPS) - 1
        )
        assert dve_ops._SUB_OPCODE_FOR_NAME[name] < 0x20
        return op

    def _opa_ref(in0, s0, s1, imm2):
        S = in0.shape[1]
        pg = (s0 + np.arange(S) * s1).astype(np.float32)[None, :, None]
        return np.maximum(imm2 - np.abs(in0 - pg), 0.0).astype(np.float32)

    def _opb_ref(in0, s0):
        p = in0.astype(np.float32)
        q = np.maximum(p - 1.0, 0.0)
        return (p * p * p - s0 * q * q * q).astype(np.float32)

    pg = PageIdx(C0, C1)
    v = Src0 - pg
    absv = maxx(v, Zero - v)
    spec_a = Spec(body=relu(C2 - absv), reference=_opa_ref)

    p = Src0
    q = relu(p - One)
    spec_b = Spec(body=sq(p) * p - sq(q) * q * C0, reference=_opb_ref)

    _OPS_CACHE["A"] = _register("KAN_ARG_ANT", spec_a, subdim=True)
    _OPS_CACHE["B"] = _register("KAN_CUBE_ANT", spec_b, subdim=False)
    return _OPS_CACHE["A"], _OPS_CACHE["B"]


# ------------------------------------------------------- bass program
_NC_CACHE = {}


def _build_nc(d_scale, d_bias, out_scale, silu_scale, general):
    """Build + compile the per-core Bass program.

    Inputs: xt (I, B_LOC) x-shard transposed; coefj (NJ, I, O) j-major coef;
            sbase (I, O); general path adds ss (I, O) and mask (I, O).
    Output: out (B_LOC, O).
    """
    key = (d_scale, d_bias, out_scale, silu_scale, general)
    if key in _NC_CACHE:
        return _NC_CACHE[key]

    import concourse.tile as tile
    from concourse import bacc, mybir

    op_a, op_b = _get_ops()
    f32 = mybir.dt.float32
    AF = mybir.ActivationFunctionType

    nc = bacc.Bacc(
        "TRN2", target_bir_lowering=False, debug=False, enable_asserts=False
    )
    xt = nc.dram_tensor("xt", (IN_DIM, B_LOC), f32, kind="ExternalInput").ap()
    coefj = nc.dram_tensor(
        "coefj", (NJ, IN_DIM, OUT_DIM), f32, kind="ExternalInput"
    ).ap()
    sbase = nc.dram_tensor("sbase", (IN_DIM, OUT_DIM), f32, kind="ExternalInput").ap()
    if general:
        ss = nc.dram_tensor("ss", (IN_DIM, OUT_DIM), f32, kind="ExternalInput").ap()
        mk = nc.dram_tensor("mask", (IN_DIM, OUT_DIM), f32, kind="ExternalInput").ap()
    out = nc.dram_tensor("out", (B_LOC, OUT_DIM), f32, kind="ExternalOutput").ap()

    with tile.TileContext(nc) as tc:
        with (
            tc.tile_pool(name="const", bufs=1) as cpool,
            tc.tile_pool(name="tmp", bufs=2) as tpool,
            tc.tile_pool(name="b6", bufs=6) as bpool,
            tc.tile_pool(name="coef", bufs=8) as wpool,
            tc.tile_pool(name="outp", bufs=2) as opool,
            tc.tile_pool(name="psum", bufs=1, space="PSUM") as pspool,
        ):
            d_tiles, silu_tiles, sb_tiles, ssm_tiles = [], [], [], []
            for it in range(N_IT):
                xt_t = cpool.tile([P, B_LOC], f32, tag=f"xt{it}")
                nc.sync.dma_start(xt_t[:], xt[it * P : (it + 1) * P, :])
                d_t = cpool.tile([P, B_LOC], f32, tag=f"d{it}")
                nc.scalar.activation(d_t[:], xt_t[:], AF.Copy, bias=d_bias, scale=d_scale)
                s_t = cpool.tile([P, B_LOC], f32, tag=f"silu{it}")
                nc.scalar.activation(s_t[:], xt_t[:], AF.Silu)
                nc.vector.tensor_scalar_mul(s_t[:], s_t[:], silu_scale)
                sb_t = cpool.tile([P, OUT_DIM], f32, tag=f"sb{it}")
                nc.sync.dma_start(sb_t[:], sbase[it * P : (it + 1) * P, :])
                if general:
                    ssm_t = cpool.tile([P, OUT_DIM], f32, tag=f"ssm{it}")
                    nc.sync.dma_start(ssm_t[:], ss[it * P : (it + 1) * P, :])
                    mk_t = cpool.tile([P, OUT_DIM], f32, tag=f"mk{it}")
                    nc.sync.dma_start(mk_t[:], mk[it * P : (it + 1) * P, :])
                    nc.vector.tensor_mul(ssm_t[:], ssm_t[:], mk_t[:])
                    nc.vector.tensor_mul(sb_t[:], sb_t[:], mk_t[:])
                    ssm_tiles.append(ssm_t)
                d_tiles.append(d_t)
                silu_tiles.append(s_t)
                sb_tiles.append(sb_t)

            psum_tiles = [
                pspool.tile([P, OUT_DIM], f32, tag=f"ps{m}", name=f"ps{m}")
                for m in range(N_M)
            ]

            # silu-residual contraction (first K tiles of the accumulation)
            for it in range(N_IT):
                for m in range(N_M):
                    nc.tensor.matmul(
                        psum_tiles[m][:],
                        silu_tiles[it][:, m * P : (m + 1) * P],
                        sb_tiles[it][:],
                        start=(it == 0),
                        stop=False,
                    )

            # spline contraction over (i, j)
            for it in range(N_IT):
                for c in range(N_CHUNK):
                    j0 = c * GJ
                    tmp = tpool.tile([P, GJ * B_LOC], f32, tag="tmp")
                    nc.vector._custom_dve(
                        op_a,
                        out=tmp[:].rearrange("p (s n) -> p s n", s=GJ),
                        in0=d_tiles[it][:, None, :].to_broadcast((P, GJ, B_LOC)),
                        s0=float(j0 + 2),
                        s1=1.0,
                        imm2=2.0,
                    )
                    b6 = bpool.tile([P, GJ * B_LOC], f32, tag="b6")
                    nc.vector._custom_dve(op_b, out=b6[:], in0=tmp[:], s0=4.0)
                    b63 = b6[:].rearrange("p (s n) -> p s n", s=GJ)
                    for jj in range(GJ):
                        j = j0 + jj
                        w_t = wpool.tile([P, OUT_DIM], f32, tag="w")
                        nc.sync.dma_start(w_t[:], coefj[j, it * P : (it + 1) * P, :])
                        if general:
                            nc.vector.tensor_mul(w_t[:], w_t[:], ssm_tiles[it][:])
                        last = (it == N_IT - 1) and (j == NJ - 1)
                        for m in range(N_M):
                            nc.tensor.matmul(
                                psum_tiles[m][:],
                                b63[:, jj, m * P : (m + 1) * P],
                                w_t[:],
                                start=False,
                                stop=last and (m == N_M - 1),
                            )

            for m in range(N_M):
                o_t = opool.tile([P, OUT_DIM], f32, tag="o")
                nc.scalar.activation(
                    o_t[:], psum_tiles[m][:], AF.Copy, scale=out_scale
                )
                nc.sync.dma_start(out[m * P : (m + 1) * P, :], o_t[:])

    nc.compile()
    _NC_CACHE[key] = nc
    return nc


# ------------------------------------------------------- numpy fallback
def _numpy_fallback(x, grid, coef, scale_base, scale_sp, mask):
    """Faithful Cox-de-Boor replica for pathological (non-uniform) grids."""
    K = 3
    xg = x[:, :, None]
    g = grid[None, :, :]
    val = ((xg >= g[:, :, :-1]) & (xg < g[:, :, 1:])).astype(x.dtype)
    for p in range(1, K + 1):
        left = (xg - g[:, :, : -(p + 1)]) / (g[:, :, p:-1] - g[:, :, : -(p + 1)])
        right = (g[:, :, p + 1 :] - xg) / (g[:, :, p + 1 :] - g[:, :, 1:-p])
        val = left * val[:, :, :-1] + right * val[:, :, 1:]
    silu = (x / (1.0 + np.exp(-x.astype(np.float64)))).astype(np.float32)
    B, I = x.shape
    O = coef.shape[1]
    W = np.ascontiguousarray(
        coef.transpose(0, 2, 1) * (scale_sp * mask)[:, None, :]
    ).reshape(I * coef.shape[2], O)
    y = val.reshape(B, -1) @ W + silu @ (scale_base * mask)
    return y.astype(np.float32)


# ------------------------------------------------------- entry point
def kernel(**inputs) -> np.ndarray:
    x = np.ascontiguousarray(np.asarray(inputs["x"], dtype=np.float32))
    grid = np.asarray(inputs["grid"], dtype=np.float32)
    coef = np.asarray(inputs["coef"], dtype=np.float32)
    scale_base = np.ascontiguousarray(np.asarray(inputs["scale_base"], dtype=np.float32))
    scale_sp = np.asarray(inputs["scale_sp"], dtype=np.float32)
    mask = np.asarray(inputs["mask"], dtype=np.float32)

    # shape guard: anything unexpected -> exact numpy fallback
    if (
        x.shape != (BATCH, IN_DIM)
        or grid.shape != (IN_DIM, 39)
        or coef.shape != (IN_DIM, OUT_DIM, NJ)
    ):
        return _numpy_fallback(x, grid, coef, scale_base, scale_sp, mask)

    # grid must be uniform + identical across rows for the shifted-bump form
    t0 = float(grid[0, 0])
    h = (float(grid[0, -1]) - t0) / 38.0
    ideal = (t0 + h * np.arange(39)).astype(np.float32)
    if h <= 0 or np.abs(grid - ideal[None, :]).max() > 1e-5 * max(1.0, abs(t0)):
        return _numpy_fallback(x, grid, coef, scale_base, scale_sp, mask)

    ss_const = float(scale_sp.flat[0])
    mk_const = float(mask.flat[0])
    fast = (
        np.all(scale_sp == ss_const)
        and np.all(mask == mk_const)
        and abs(ss_const * mk_const) > 1e-30
    )

    if fast:
        out_scale = ss_const * mk_const / 6.0
        silu_scale = mk_const / out_scale  # = 6 / ss_const
    else:
        out_scale = 1.0 / 6.0
        silu_scale = 6.0

    nc = _build_nc(1.0 / h, -t0 / h, out_scale, silu_scale, not fast)

    coefj = np.ascontiguousarray(coef.transpose(2, 0, 1))
    in_maps = []
    for c in range(N_CORES):
        m = {
            "xt": np.ascontiguousarray(x[c * B_LOC : (c + 1) * B_LOC, :].T),
            "coefj": coefj,
            "sbase": scale_base,
        }
        if not fast:
            m["ss"] = np.ascontiguousarray(scale_sp)
            m["mask"] = np.ascontiguousarray(mask)
        in_maps.append(m)

    from concourse.bass_utils import run_bass_kernel_spmd

    res = run_bass_kernel_spmd(nc, in_maps, core_ids=list(range(N_CORES)))
    return np.concatenate([r["out"] for r in res.results], axis=0)


# revision 9
# speedup vs baseline: 82922.1353x; 82922.1353x over previous
"""KAN layer (silu residual + cubic B-spline mixing) on 8 Trainium2 cores.

Math: out(b,o) = sum_i [ scale_base*mask*silu(x) + scale_sp*mask*sum_j N_j(x)*coef ]
The grid is uniform, so the 35 spline basis functions are shifts of ONE
cubic bump N3 on knots {0..4}:  bases_j(b,i) = N3(d(b,i) - j),
d = (x - t0)/h.  Cancellation-free closed form used on-device:

    p  = relu(2 - |d - j - 2|)          (custom DVE op A, PageIdx supplies j)
    6*N3 = p^3 - 4*relu(p-1)^3          (custom DVE op B)

The whole layer then becomes one dense matmul with contraction over
(i, j) [8960] plus a silu-residual contraction over i [256]:
    out = S * [ sum_{i,j} B6[(i,j),b]^T @ coef[(i,j),o] + silu_scaled^T @ scale_base ]

Sharding: data-parallel over batch; 8 cores x 256 rows each. coef and
scale_base replicated. Inside each core: i on partitions for the basis
build (lhsT layout, no transposes anywhere), PSUM accumulates (b,o).
"""

import sys

if "/opt/trn_rl_repo" not in sys.path:
    sys.path.insert(0, "/opt/trn_rl_repo")

import numpy as np

# ----------------------------------------------------------------- constants
P = 128
IN_DIM = 256
OUT_DIM = 512
BATCH = 2048
N_CORES = 8
B_LOC = BATCH // N_CORES  # 256
NJ = 35  # spline basis functions (G + K)
GJ = 7  # j-chunk size for DVE/PE pipelining
N_CHUNK = NJ // GJ  # 5
N_IT = IN_DIM // P  # 2 partition tiles of in_dim
N_M = B_LOC // P  # 2 output row tiles

# ------------------------------------------------------- custom DVE ops
_OPS_CACHE = {}


def _get_ops():
    if _OPS_CACHE:
        return _OPS_CACHE["A"], _OPS_CACHE["B"]
    import concourse.dve_ops as dve_ops
    from concourse.dve_ops import DveOp, OPS
    from concourse.dve_spec import (
        C0,
        C1,
        C2,
        PageIdx,
        Spec,
        Src0,
        Zero,
        One,
        _has_src1,
        lower,
        maxx,
        relu,
        sq,
    )
    from concourse.dve_uop import DveOpSpec

    def _register(name, spec, subdim):
        for op in OPS:
            if op.name == name:
                return op
        shas = {}
        for ver in ("v3", "v4"):
            tmp = DveOpSpec(
                name=name, opcode=1, uops=lower(spec, ver=ver), rd1_en=_has_src1(spec)
            )
            shas[ver] = tmp.sha(ver)
        op = DveOp(name, spec, subdim=subdim, uops_sha=shas)
        OPS.append(op)
        dve_ops.CUSTOM_DVE_SPECS[name] = spec
        dve_ops._SUB_OPCODE_FOR_NAME[name] = (
            dve_ops._CUSTOM_DVE_ROW_BASE + len(OPS) - 1
        )
        assert dve_ops._SUB_OPCODE_FOR_NAME[name] < 0x20
        return op

    def _opa_ref(in0, s0, s1, imm2):
        S = in0.shape[1]
        pg = (s0 + np.arange(S) * s1).astype(np.float32)[None, :, None]
        return np.maximum(imm2 - np.abs(in0 - pg), 0.0).astype(np.float32)

    def _opb_ref(in0, s0):
        p = in0.astype(np.float32)
        q = np.maximum(p - 1.0, 0.0)
        return (p * p * p - s0 * q * q * q).astype(np.float32)

    pg = PageIdx(C0, C1)
    v = Src0 - pg
    absv = maxx(v, Zero - v)
    spec_a = Spec(body=relu(C2 - absv), reference=_opa_ref)

    p = Src0
    q = relu(p - One)
    spec_b = Spec(body=sq(p) * p - sq(q) * q * C0, reference=_opb_ref)

    _OPS_CACHE["A"] = _register("KAN_ARG_ANT", spec_a, subdim=True)
    _OPS_CACHE["B"] = _register("KAN_CUBE_ANT", spec_b, subdim=False)
    return _OPS_CACHE["A"], _OPS_CACHE["B"]


# ------------------------------------------------------- bass program
_NC_CACHE = {}


def _build_nc(d_scale, d_bias, out_scale, silu_scale, general):
    """Build + compile the per-core Bass program.

    Fast path: coefi (N_IT, 128, NJ, O) bf16, i-major so each chunk DMA is
    128 long contiguous runs; spline matmuls run bf16 (4x faster PE than
    fp32), silu-residual matmuls stay fp32.
    General path: coefj (NJ, I, O) fp32 with on-device scale_sp*mask scaling.
    Output: out (B_LOC, O) fp32.
    """
    key = (d_scale, d_bias, out_scale, silu_scale, general)
    if key in _NC_CACHE:
        return _NC_CACHE[key]

    import concourse.tile as tile
    from concourse import bacc, mybir

    op_a, op_b = _get_ops()
    f32 = mybir.dt.float32
    bf16 = mybir.dt.bfloat16
    AF = mybir.ActivationFunctionType

    nc = bacc.Bacc(
        "TRN2", target_bir_lowering=False, debug=False, enable_asserts=False
    )
    xt = nc.dram_tensor("xt", (IN_DIM, B_LOC), f32, kind="ExternalInput").ap()
    if general:
        coefj = nc.dram_tensor(
            "coefj", (NJ, IN_DIM, OUT_DIM), f32, kind="ExternalInput"
        ).ap()
    else:
        coefi = nc.dram_tensor(
            "coefi", (N_IT, P, NJ, OUT_DIM), bf16, kind="ExternalInput"
        ).ap()
    sbase = nc.dram_tensor("sbase", (IN_DIM, OUT_DIM), f32, kind="ExternalInput").ap()
    if general:
        ss = nc.dram_tensor("ss", (IN_DIM, OUT_DIM), f32, kind="ExternalInput").ap()
        mk = nc.dram_tensor("mask", (IN_DIM, OUT_DIM), f32, kind="ExternalInput").ap()
    out = nc.dram_tensor("out", (B_LOC, OUT_DIM), f32, kind="ExternalOutput").ap()

    with tile.TileContext(nc) as tc:
        with (
            tc.tile_pool(name="const", bufs=1) as cpool,
            tc.tile_pool(name="tmp", bufs=2) as tpool,
            tc.tile_pool(name="b6", bufs=6) as bpool,
            tc.tile_pool(name="coef", bufs=5) as wpool,
            tc.tile_pool(name="outp", bufs=2) as opool,
            tc.tile_pool(name="psum", bufs=1, space="PSUM") as pspool,
        ):
            d_tiles, silu_tiles, sb_tiles, ssm_tiles = [], [], [], []
            for it in range(N_IT):
                xt_t = cpool.tile([P, B_LOC], f32, tag=f"xt{it}")
                nc.sync.dma_start(xt_t[:], xt[it * P : (it + 1) * P, :])
                d_t = cpool.tile([P, B_LOC], f32, tag=f"d{it}")
                nc.scalar.activation(d_t[:], xt_t[:], AF.Copy, bias=d_bias, scale=d_scale)
                s_t = cpool.tile([P, B_LOC], f32, tag=f"silu{it}")
                nc.scalar.activation(s_t[:], xt_t[:], AF.Silu)
                nc.vector.tensor_scalar_mul(s_t[:], s_t[:], silu_scale)
                sb_t = cpool.tile([P, OUT_DIM], f32, tag=f"sb{it}")
                nc.sync.dma_start(sb_t[:], sbase[it * P : (it + 1) * P, :])
                if general:
                    ssm_t = cpool.tile([P, OUT_DIM], f32, tag=f"ssm{it}")
                    nc.sync.dma_start(ssm_t[:], ss[it * P : (it + 1) * P, :])
                    mk_t = cpool.tile([P, OUT_DIM], f32, tag=f"mk{it}")
                    nc.sync.dma_start(mk_t[:], mk[it * P : (it + 1) * P, :])
                    nc.vector.tensor_mul(ssm_t[:], ssm_t[:], mk_t[:])
                    nc.vector.tensor_mul(sb_t[:], sb_t[:], mk_t[:])
                    ssm_tiles.append(ssm_t)
                d_tiles.append(d_t)
                silu_tiles.append(s_t)
                sb_tiles.append(sb_t)

            psum_tiles = [
                pspool.tile([P, OUT_DIM], f32, tag=f"ps{m}", name=f"ps{m}")
                for m in range(N_M)
            ]

            # silu-residual contraction (first K tiles of the accumulation)
            for it in range(N_IT):
                for m in range(N_M):
                    nc.tensor.matmul(
                        psum_tiles[m][:],
                        silu_tiles[it][:, m * P : (m + 1) * P],
                        sb_tiles[it][:],
                        start=(it == 0),
                        stop=False,
                    )

            # spline contraction over (i, j)
            mm_dt = f32 if general else bf16
            for it in range(N_IT):
                for c in range(N_CHUNK):
                    j0 = c * GJ
                    tmp = tpool.tile([P, GJ * B_LOC], f32, tag="tmp")
                    nc.vector._custom_dve(
                        op_a,
                        out=tmp[:].rearrange("p (s n) -> p s n", s=GJ),
                        in0=d_tiles[it][:, None, :].to_broadcast((P, GJ, B_LOC)),
                        s0=float(j0 + 2),
                        s1=1.0,
                        imm2=2.0,
                    )
                    b6 = bpool.tile([P, GJ * B_LOC], mm_dt, tag="b6")
                    nc.vector._custom_dve(op_b, out=b6[:], in0=tmp[:], s0=4.0)
                    b63 = b6[:].rearrange("p (s n) -> p s n", s=GJ)

                    if general:
                        w_c = None
                    else:
                        w_c = wpool.tile([P, GJ * OUT_DIM], bf16, tag="w")
                        nc.sync.dma_start(
                            w_c[:].rearrange("p (j o) -> p j o", j=GJ),
                            coefi[it, :, j0 : j0 + GJ, :],
                        )
                    for jj in range(GJ):
                        j = j0 + jj
                        if general:
                            w_t = wpool.tile([P, OUT_DIM], f32, tag="wg")
                            nc.sync.dma_start(
                                w_t[:], coefj[j, it * P : (it + 1) * P, :]
                            )
                            nc.vector.tensor_mul(w_t[:], w_t[:], ssm_tiles[it][:])
                            rhs = w_t[:]
                        else:
                            rhs = w_c[:, jj * OUT_DIM : (jj + 1) * OUT_DIM]
                        last = (it == N_IT - 1) and (j == NJ - 1)
                        for m in range(N_M):
                            nc.tensor.matmul(
                                psum_tiles[m][:],
                                b63[:, jj, m * P : (m + 1) * P],
                                rhs,
                                start=False,
                                stop=last and (m == N_M - 1),
                            )

            for m in range(N_M):
                o_t = opool.tile([P, OUT_DIM], f32, tag="o")
                nc.scalar.activation(
                    o_t[:], psum_tiles[m][:], AF.Copy, scale=out_scale
                )
                nc.sync.dma_start(out[m * P : (m + 1) * P, :], o_t[:])

    nc.compile()
    _NC_CACHE[key] = nc
    return nc


# ------------------------------------------------------- numpy fallback
def _numpy_fallback(x, grid, coef, scale_base, scale_sp, mask):
    """Faithful Cox-de-Boor replica for pathological (non-uniform) grids."""
    K = 3
    xg = x[:, :, None]
    g = grid[None, :, :]
    val = ((xg >= g[:, :, :-1]) & (xg < g[:, :, 1:])).astype(x.dtype)
    for p in range(1, K + 1):
        left = (xg - g[:, :, : -(p + 1)]) / (g[:, :, p:-1] - g[:, :, : -(p + 1)])
        right = (g[:, :, p + 1 :] - xg) / (g[:, :, p + 1 :] - g[:, :, 1:-p])
        val = left * val[:, :, :-1] + right * val[:, :, 1:]
    silu = (x / (1.0 + np.exp(-x.astype(np.float64)))).astype(np.float32)
    B, I = x.shape
    O = coef.shape[1]
    W = np.ascontiguousarray(
        coef.transpose(0, 2, 1) * (scale_sp * mask)[:, None, :]
    ).reshape(I * coef.shape[2], O)
    y = val.reshape(B, -1) @ W + silu @ (scale_base * mask)
    return y.astype(np.float32)


# ------------------------------------------------------- entry point
def _setup(**inputs):
    """Returns ("fallback", out) or (nc, fast, x, coef, scale_base, ss, mask)."""
    x = np.ascontiguousarray(np.asarray(inputs["x"], dtype=np.float32))
    grid = np.asarray(inputs["grid"], dtype=np.float32)
    coef = np.asarray(inputs["coef"], dtype=np.float32)
    scale_base = np.ascontiguousarray(np.asarray(inputs["scale_base"], dtype=np.float32))
    scale_sp = np.asarray(inputs["scale_sp"], dtype=np.float32)
    mask = np.asarray(inputs["mask"], dtype=np.float32)

    # shape guard: anything unexpected -> exact numpy fallback
    if (
        x.shape != (BATCH, IN_DIM)
        or grid.shape != (IN_DIM, 39)
        or coef.shape != (IN_DIM, OUT_DIM, NJ)
    ):
        return "fallback", _numpy_fallback(x, grid, coef, scale_base, scale_sp, mask)

    # grid must be uniform + identical across rows for the shifted-bump form
    t0 = float(grid[0, 0])
    h = (float(grid[0, -1]) - t0) / 38.0
    ideal = (t0 + h * np.arange(39)).astype(np.float32)
    if h <= 0 or np.abs(grid - ideal[None, :]).max() > 1e-5 * max(1.0, abs(t0)):
        return "fallback", _numpy_fallback(x, grid, coef, scale_base, scale_sp, mask)

    ss_const = float(scale_sp.flat[0])
    mk_const = float(mask.flat[0])
    fast = (
        np.all(scale_sp == ss_const)
        and np.all(mask == mk_const)
        and abs(ss_const * mk_const) > 1e-30
    )

    if fast:
        out_scale = ss_const * mk_const / 6.0
        silu_scale = mk_const / out_scale  # = 6 / ss_const
    else:
        out_scale = 1.0 / 6.0
        silu_scale = 6.0

    nc = _build_nc(1.0 / h, -t0 / h, out_scale, silu_scale, not fast)
    return nc, fast, x, coef, scale_base, scale_sp, mask


def _prep(x, coef, scale_base, scale_sp, mask, fast):
    if fast:
        import ml_dtypes

        # (I, O, NJ) -> (N_IT, 128, NJ, O), bf16, i-major contiguous runs
        coef_in = np.ascontiguousarray(
            coef.reshape(N_IT, P, OUT_DIM, NJ).transpose(0, 1, 3, 2)
        ).astype(ml_dtypes.bfloat16)
        coef_key = "coefi"
    else:
        coef_in = np.ascontiguousarray(coef.transpose(2, 0, 1))
        coef_key = "coefj"
    in_maps = []
    for c in range(N_CORES):
        m = {
            "xt": np.ascontiguousarray(x[c * B_LOC : (c + 1) * B_LOC, :].T),
            coef_key: coef_in,
            "sbase": scale_base,
        }
        if not fast:
            m["ss"] = np.ascontiguousarray(scale_sp)
            m["mask"] = np.ascontiguousarray(mask)
        in_maps.append(m)
    return in_maps


def kernel(**inputs) -> np.ndarray:
    r = _setup(**inputs)
    if r[0] == "fallback":
        return r[1]
    nc, fast, x, coef, scale_base, scale_sp, mask = r
    in_maps = _prep(x, coef, scale_base, scale_sp, mask, fast)

    from concourse.bass_utils import run_bass_kernel_spmd

    res = run_bass_kernel_spmd(nc, in_maps, core_ids=list(range(N_CORES)))
    return np.concatenate([r["out"] for r in res.results], axis=0)
